# revision 9
# baseline (speedup 1.0000x reference)
"""Trainium2 Bass kernel for nn_Conv2d (B=32, Cin=Cout=64, H=W=112, 3x3, pad 1).

Strategy:
- Data-parallel: 32 images / 8 cores = 4 images per core; weights/bias replicated.
- Host pre-pads each image to a flat zero-padded 114x114 buffer so every input
  DMA is one contiguous 52KB run per channel (64 descriptors per image instead
  of 28k 448B row-runs) -- the PE starts ~8us earlier and queues stay free.
- Per core: 2 image-PAIRS. Image A on SBUF partitions 0-63, image B on 64-127.
- Conv = 9 accumulating PE matmuls per 456-pixel chunk: for tap (r,c) the
  stationary lhsT is a 128x128 block-diagonal tile diag(w_rc^T, w_rc^T); the
  moving rhs is the padded image buffer at free-dim offset r*114+c. PSUM (fp32)
  accumulates all 9 taps.
- PE pstate warm-up: a chain of dummy matmuls issued before any data dependency
  so the 0.65/1.2GHz clock ramp happens inside the DMA-head shadow and real
  matmuls run at full clock from the first instruction.
- Epilogue: VectorE tensor_scalar_add(psum + per-partition bias) -> SBUF staging,
  then tapered output row-blocks round-robined over the Scalar/Sync/GpSimd DMA
  queues (small final blocks shrink the drain tail).
- dtype float32r: fp32 storage, fast PE mode (1 cycle/row at N>=256),
  measured ~1.5e-4 max relative error end-to-end.
"""
import numpy as np

B, CIN, COUT, H, W = 32, 64, 64, 112, 112
N_CORES = 8
IPC = B // N_CORES          # images per core = 4
NPAIR = IPC // 2            # image pairs per core = 2
Wp = W + 2                  # padded width 114
Hp = H + 2                  # padded height 114
ROWS_PER_CHUNK = 4
CH = ROWS_PER_CHUNK * Wp    # chunk size 456 (row-aligned; fits one PSUM bank)
NCHUNK = H // ROWS_PER_CHUNK     # 28 chunks cover all 112 output rows exactly
LB = Hp * Wp + 256          # padded-image buffer length (+tail margin for taps)
N_WARM = 18                 # PE pstate warm-up matmuls (~3.5us of ramp)

# input row-block schedule (padded rows); small slivers first so chunk-0
# matmuls start almost immediately
IN_BLOCKS = [(0, 7), (7, 7), (14, 14), (28, 28), (56, 28), (84, 30)]
# output row-block schedule: 14-row blocks, tapering at the end so the last
# store (the drain tail) is small
OUT_BLOCKS = [(0, 14), (14, 14), (28, 14), (42, 14), (56, 14), (70, 14),
              (84, 14), (98, 7), (105, 7)]

_CACHE = {}


def _build_module():
    import concourse.tile as tile
    from concourse import bacc, mybir
    from concourse.bass_interp import get_hw_module

    f32 = mybir.dt.float32
    f32r = mybir.dt.float32r

    nc = bacc.Bacc("TRN2", target_bir_lowering=False, debug=False,
                   enable_asserts=False, num_devices=N_CORES)
    x_ap = nc.dram_tensor("xp", [IPC, CIN, LB], f32r, kind="ExternalInput").ap()
    wt_ap = nc.dram_tensor("wt", [128, 9 * 128], f32r, kind="ExternalInput").ap()
    b_ap = nc.dram_tensor("bias2", [128, 1], f32, kind="ExternalInput").ap()
    y_ap = nc.dram_tensor("y", [IPC, COUT, H, W], f32, kind="ExternalOutput").ap()

    with tile.TileContext(nc) as tc:
        with (
            tc.tile_pool(name="const", bufs=1) as cp,
            tc.tile_pool(name="psum", bufs=8, space="PSUM") as pp,
        ):
            # Every dma_start costs ~650ns of serialized DIRECT2D issue time
            # on its engine, so the pre-loop program carries only what the
            # first chunk needs (weights, bias, pair0 block0/1); all later
            # input blocks are issued from inside the main loop, well before
            # their consuming chunks.
            w_sb = cp.tile([128, 9 * 128], f32r)
            w_qs = [nc.scalar, nc.sync, nc.gpsimd]
            x2s = [cp.tile([128, LB], f32r, name=f"x2_{k}", tag=f"x2_{k}")
                   for k in range(NPAIR)]
            oimg = cp.tile([128, H * W], f32)

            def load_block(p, b):
                rb, nr = IN_BLOCKS[b]
                lo, hi = rb * Wp, min((rb + nr) * Wp, LB)
                for h in range(2):
                    img = 2 * p + h
                    eng = nc.sync if h == 0 else nc.gpsimd
                    eng.dma_start(x2s[p][64 * h:64 * (h + 1), lo:hi],
                                  x_ap[img, :, lo:hi])

            nc.scalar.dma_start(w_sb[:, 0:128], wt_ap[:, 0:128])   # tap 0
            nc.sync.dma_start(w_sb[:, 128:256], wt_ap[:, 128:256])
            nc.gpsimd.dma_start(w_sb[:, 256:384], wt_ap[:, 256:384])
            load_block(0, 0)
            for t in range(3, 9):
                w_qs[t % 3].dma_start(w_sb[:, t * 128:(t + 1) * 128],
                                      wt_ap[:, t * 128:(t + 1) * 128])
            load_block(0, 1)
            bias_sb = cp.tile([128, 1], f32)
            nc.scalar.dma_start(bias_sb[:], b_ap[:])

            # (pair, block) -> issue just before this (pair, chunk) runs
            PREFETCH = {(0, 0): [(0, 2)], (0, 1): [(0, 3)], (0, 4): [(0, 4)],
                        (0, 10): [(0, 5)], (0, 16): [(1, 0)], (0, 18): [(1, 1)],
                        (0, 20): [(1, 2)], (0, 22): [(1, 3)], (1, 2): [(1, 4)],
                        (1, 8): [(1, 5)]}

            # ---- main loop: 9 accumulating taps per 456-col chunk ----
            out_qs = [nc.scalar, nc.sync, nc.gpsimd]
            qi = 0
            for p in range(NPAIR):
                x2 = x2s[p]
                nxt = 0  # index into OUT_BLOCKS
                for c in range(NCHUNK):
                    for pb in PREFETCH.get((p, c), []):
                        load_block(*pb)
                    s = c * CH
                    ps = pp.tile([128, CH], f32)
                    for t in range(9):
                        r, cc = divmod(t, 3)
                        off = r * Wp + cc
                        nc.tensor.matmul(ps[:], w_sb[:, t * 128:(t + 1) * 128],
                                         x2[:, s + off:s + off + CH],
                                         start=(t == 0), stop=(t == 8))
                    # keep the 112 valid columns of each padded row, add bias,
                    # write contiguous HBM layout
                    pv = ps[:].rearrange("p (h w) -> p h w", w=Wp)[:, :, 0:W]
                    ov = oimg[:, c * ROWS_PER_CHUNK * W:(c + 1) * ROWS_PER_CHUNK * W]
                    nc.vector.tensor_scalar_add(
                        ov.rearrange("p (h w) -> p h w", w=W), pv, bias_sb[:])
                    # emit output row-blocks as soon as their rows are evacuated
                    while nxt < len(OUT_BLOCKS) and \
                            sum(OUT_BLOCKS[nxt]) <= (c + 1) * ROWS_PER_CHUNK:
                        rb0, nr0 = OUT_BLOCKS[nxt]
                        for h in range(2):
                            img = 2 * p + h
                            src = oimg[64 * h:64 * (h + 1),
                                       W * rb0:W * (rb0 + nr0)]
                            out_qs[qi % 3].dma_start(
                                y_ap[img, :, rb0:rb0 + nr0, :],
                                src.rearrange("p (h w) -> p h w", w=W))
                            qi += 1
                        nxt += 1

    nc.compile()
    nc.m = get_hw_module(nc.m)
    return nc


def _get_module():
    if "nc" not in _CACHE:
        _CACHE["nc"] = _build_module()
    return _CACHE["nc"]


def _make_in_maps(x, weight, bias):
    x = np.asarray(x, np.float32)
    # flat zero-padded 114x114 image buffers (+ tap-overread margin)
    xp = np.zeros((B, CIN, LB), np.float32)
    xpv = xp[:, :, :Hp * Wp].reshape(B, CIN, Hp, Wp)
    xpv[:, :, 1:1 + H, 1:1 + W] = x
    # block-diagonal per-tap weights in SBUF layout [cin_k, tap, cout_m]
    wt = np.zeros((128, 9, 128), np.float32)
    for t in range(9):
        r, cc = divmod(t, 3)
        wT = np.ascontiguousarray(weight[:, :, r, cc].T)  # [cin, cout]
        wt[:64, t, :64] = wT
        wt[64:, t, 64:] = wT
    wt = np.ascontiguousarray(wt.reshape(128, 9 * 128))
    bias2 = np.tile(np.asarray(bias, np.float32).reshape(COUT, 1), (2, 1))
    return [{"xp": np.ascontiguousarray(xp[c * IPC:(c + 1) * IPC]),
             "wt": wt, "bias2": bias2} for c in range(N_CORES)]


def _run(in_maps, trace=False):
    from concourse import bass_utils
    nc = _get_module()
    return bass_utils.run_bass_kernel_spmd(
        nc, in_maps, core_ids=list(range(N_CORES)), trace=trace)


def kernel(x, weight, bias):
    res = _run(_make_in_maps(x, weight, bias), trace=False)
    return np.concatenate([res.results[c]["y"] for c in range(N_CORES)], axis=0)


# revision 10
# speedup vs baseline: 1.1161x; 1.1161x over previous
"""Trainium2 Bass kernel for nn_Conv2d (B=32, Cin=Cout=64, H=W=112, 3x3, pad 1).

Strategy:
- Data-parallel: 32 images / 8 cores = 4 images per core; weights/bias replicated.
- bf16 inputs + weights (PE runs bf16 at the same 1 cycle/row as fp32r, rel err
  ~1e-3 vs the 2e-2 gate): halves input HBM traffic, halves the head-critical
  bytes (weights + first row-blocks) that gate the first matmul, and shrinks
  the per-matmul LDWEIGHTS pipeline. PSUM/bias/output stay fp32.
- Host pre-pads each image to a flat zero-padded 114x114 bf16 buffer so every
  input DMA is one contiguous run per channel (64 descriptors per image-block).
- Weights ship compact ([128, 9x64] = 148KB instead of the 590KB block-diag);
  the 128x128 block-diagonal per-tap tiles are built on-chip with one memset +
  4 VectorE copies.
- Per core: 2 image-PAIRS. Image A on SBUF partitions 0-63, image B on 64-127.
- Conv = 9 accumulating PE matmuls per 456-pixel chunk: for tap (r,c) the
  stationary lhsT is the 128x128 block-diag diag(w_rc^T, w_rc^T); the moving
  rhs is the padded image buffer at free-dim offset r*114+c. PSUM f32
  accumulates all 9 taps.
- Every dma_start costs ~650ns of serialized DIRECT2D issue time on its
  engine, so the pre-loop program carries only what the first chunks need;
  later input blocks are issued from inside the main loop well ahead of use.
- Epilogue: VectorE tensor_scalar_add(psum + bias) -> f32 SBUF staging, then
  tapered output row-blocks round-robined over Scalar/Sync/GpSimd DMA queues.
"""
import numpy as np

B, CIN, COUT, H, W = 32, 64, 64, 112, 112
N_CORES = 8
IPC = B // N_CORES          # images per core = 4
NPAIR = IPC // 2            # image pairs per core = 2
Wp = W + 2                  # padded width 114
Hp = H + 2                  # padded height 114
ROWS_PER_CHUNK = 4
CH = ROWS_PER_CHUNK * Wp    # chunk size 456 (row-aligned; fits one PSUM bank)
NCHUNK = H // ROWS_PER_CHUNK     # 28 chunks cover all 112 output rows exactly
LB = Hp * Wp + 256          # padded-image buffer length (+tap-overread margin)

# input row-block schedule (padded rows); small slivers first so chunk-0
# matmuls start almost immediately
IN_BLOCKS = [(0, 7), (7, 7), (14, 14), (28, 28), (56, 28), (84, 30)]
# (pair, chunk) -> input blocks to issue just before that chunk runs
PREFETCH = {(0, 8): [(0, 4)], (0, 14): [(0, 5)], (0, 18): [(1, 0)],
            (0, 21): [(1, 1)], (0, 24): [(1, 2)], (0, 27): [(1, 3)],
            (1, 4): [(1, 4)], (1, 10): [(1, 5)]}
# output row-block schedule: 14-row blocks tapering at the end (small drain)
OUT_BLOCKS = [(0, 14), (14, 14), (28, 14), (42, 14), (56, 14), (70, 14),
              (84, 14), (98, 7), (105, 7)]

_CACHE = {}


def _build_module():
    import concourse.tile as tile
    from concourse import bacc, mybir
    from concourse.bass_interp import get_hw_module

    f32 = mybir.dt.float32
    bf16 = mybir.dt.bfloat16

    nc = bacc.Bacc("TRN2", target_bir_lowering=False, debug=False,
                   enable_asserts=False, num_devices=N_CORES)
    x_ap = nc.dram_tensor("xp", [IPC, CIN, LB], bf16, kind="ExternalInput").ap()
    wc_ap = nc.dram_tensor("wc", [128, 9 * 64], bf16, kind="ExternalInput").ap()
    b_ap = nc.dram_tensor("bias2", [128, 1], f32, kind="ExternalInput").ap()
    y_ap = nc.dram_tensor("y", [IPC, COUT, H, W], f32, kind="ExternalOutput").ap()

    with tile.TileContext(nc) as tc:
        with (
            tc.tile_pool(name="const", bufs=1) as cp,
            tc.tile_pool(name="psum", bufs=8, space="PSUM") as pp,
        ):
            w_sb = cp.tile([128, 9 * 128], bf16)
            wc = cp.tile([128, 9 * 64], bf16)
            x2s = [cp.tile([128, LB], bf16, name=f"x2_{k}", tag=f"x2_{k}")
                   for k in range(NPAIR)]
            oimg = cp.tile([128, H * W], f32)
            bias_sb = cp.tile([128, 1], f32)

            def load_block(p, b):
                rb, nr = IN_BLOCKS[b]
                lo, hi = rb * Wp, min((rb + nr) * Wp, LB)
                for h in range(2):
                    img = 2 * p + h
                    eng = nc.sync if h == 0 else nc.gpsimd
                    eng.dma_start(x2s[p][64 * h:64 * (h + 1), lo:hi],
                                  x_ap[img, :, lo:hi])

            # zero the off-diagonal weight blocks once (bf16 tile via f32 view)
            nc.vector.memset(w_sb[:].bitcast(f32), 0.0)
            # compact weights: tap 0 first, then taps 1-8
            nc.scalar.dma_start(wc[:, 0:64], wc_ap[:, 0:64])
            load_block(0, 0)
            nc.scalar.dma_start(wc[:, 64:9 * 64], wc_ap[:, 64:9 * 64])
            load_block(0, 1)
            nc.scalar.dma_start(bias_sb[:], b_ap[:])
            load_block(0, 2)
            load_block(0, 3)

            # build the block-diagonal: w_sb[0:64, t*128:t*128+64]   = wT_t
            #                           w_sb[64:128, t*128+64:+128] = wT_t
            nc.vector.tensor_copy(w_sb[0:64, 0:64], wc[0:64, 0:64])
            nc.vector.tensor_copy(w_sb[64:128, 64:128], wc[64:128, 0:64])
            wv_top = w_sb[0:64, 128:9 * 128].rearrange(
                "p (t m) -> p t m", m=128)[:, :, 0:64]
            wv_bot = w_sb[64:128, 128:9 * 128].rearrange(
                "p (t m) -> p t m", m=128)[:, :, 64:128]
            wcr_top = wc[0:64, 64:9 * 64].rearrange("p (t m) -> p t m", m=64)
            wcr_bot = wc[64:128, 64:9 * 64].rearrange("p (t m) -> p t m", m=64)
            nc.vector.tensor_copy(wv_top, wcr_top)
            nc.vector.tensor_copy(wv_bot, wcr_bot)

            # ---- main loop: 9 accumulating taps per 456-col chunk ----
            out_qs = [nc.scalar, nc.sync, nc.gpsimd]
            qi = 0
            for p in range(NPAIR):
                x2 = x2s[p]
                nxt = 0  # index into OUT_BLOCKS
                for c in range(NCHUNK):
                    for pb in PREFETCH.get((p, c), []):
                        load_block(*pb)
                    s = c * CH
                    ps = pp.tile([128, CH], f32)
                    for t in range(9):
                        r, cc = divmod(t, 3)
                        off = r * Wp + cc
                        nc.tensor.matmul(ps[:], w_sb[:, t * 128:(t + 1) * 128],
                                         x2[:, s + off:s + off + CH],
                                         start=(t == 0), stop=(t == 8))
                    # keep the 112 valid columns of each padded row, add bias,
                    # write contiguous HBM layout
                    pv = ps[:].rearrange("p (h w) -> p h w", w=Wp)[:, :, 0:W]
                    ov = oimg[:, c * ROWS_PER_CHUNK * W:(c + 1) * ROWS_PER_CHUNK * W]
                    nc.vector.tensor_scalar_add(
                        ov.rearrange("p (h w) -> p h w", w=W), pv, bias_sb[:])
                    # emit output row-blocks as soon as their rows are evacuated
                    while nxt < len(OUT_BLOCKS) and \
                            sum(OUT_BLOCKS[nxt]) <= (c + 1) * ROWS_PER_CHUNK:
                        rb0, nr0 = OUT_BLOCKS[nxt]
                        for h in range(2):
                            img = 2 * p + h
                            src = oimg[64 * h:64 * (h + 1),
                                       W * rb0:W * (rb0 + nr0)]
                            out_qs[qi % 3].dma_start(
                                y_ap[img, :, rb0:rb0 + nr0, :],
                                src.rearrange("p (h w) -> p h w", w=W))
                            qi += 1
                        nxt += 1

    nc.compile()
    nc.m = get_hw_module(nc.m)
    return nc


def _get_module():
    if "nc" not in _CACHE:
        _CACHE["nc"] = _build_module()
    return _CACHE["nc"]


def _to_bf16(a):
    import ml_dtypes
    return np.asarray(a, np.float32).astype(ml_dtypes.bfloat16)


def _make_in_maps(x, weight, bias):
    import ml_dtypes
    x = np.asarray(x, np.float32)
    # flat zero-padded 114x114 bf16 image buffers (+ tap-overread margin)
    xp = np.zeros((B, CIN, LB), ml_dtypes.bfloat16)
    xpv = xp[:, :, :Hp * Wp].reshape(B, CIN, Hp, Wp)
    xpv[:, :, 1:1 + H, 1:1 + W] = x.astype(ml_dtypes.bfloat16)
    # compact per-tap weights [k, t, m]: wT_t duplicated on both partition
    # halves (the on-chip copies place them block-diagonally)
    wc = np.zeros((128, 9, 64), np.float32)
    for t in range(9):
        r, cc = divmod(t, 3)
        wT = weight[:, :, r, cc].T  # [cin, cout]
        wc[:64, t, :] = wT
        wc[64:, t, :] = wT
    wc = np.ascontiguousarray(
        wc.reshape(128, 9 * 64)).astype(ml_dtypes.bfloat16)
    bias2 = np.tile(np.asarray(bias, np.float32).reshape(COUT, 1), (2, 1))
    return [{"xp": np.ascontiguousarray(xp[c * IPC:(c + 1) * IPC]),
             "wc": wc, "bias2": bias2} for c in range(N_CORES)]


def _run(in_maps, trace=False):
    from concourse import bass_utils
    nc = _get_module()
    return bass_utils.run_bass_kernel_spmd(
        nc, in_maps, core_ids=list(range(N_CORES)), trace=trace)


def kernel(x, weight, bias):
    res = _run(_make_in_maps(x, weight, bias), trace=False)
    return np.concatenate([res.results[c]["y"] for c in range(N_CORES)], axis=0)
